# revision 2
# baseline (speedup 1.0000x reference)
import numpy as np
import ml_dtypes

B, H, N, D = 4, 12, 8192, 64
M = 128
NCORES = 8
PAIRS = (B * H) // NCORES
NCHUNK = 512
NCH = N // NCHUNK
NT = N // 128

_cache = {}


def _build():
    if "nc" in _cache:
        return _cache["nc"]
    import concourse.bacc as bacc
    import concourse.mybir as mybir
    import concourse.tile as tile

    f32, f32r, bf16 = mybir.dt.float32, mybir.dt.float32r, mybir.dt.bfloat16
    AF = mybir.ActivationFunctionType

    nc = bacc.Bacc("TRN2", target_bir_lowering=False, debug=False)
    QT = nc.declare_dram_parameter("QT", [PAIRS, 64, N], f32, isOutput=False)
    KT = nc.declare_dram_parameter("KT", [PAIRS, 64, N], f32, isOutput=False)
    Vb = nc.declare_dram_parameter("Vb", [PAIRS, N, 64], bf16, isOutput=False)
    NCT = nc.declare_dram_parameter("NCT", [PAIRS, 64, M], f32, isOutput=False)
    NRT = nc.declare_dram_parameter("NRT", [PAIRS, 64, M], f32, isOutput=False)
    GS = nc.declare_dram_parameter("GS", [1, 1], f32, isOutput=False)
    XO = nc.declare_dram_parameter("XO", [PAIRS, N, 64], f32, isOutput=True)

    with tile.TileContext(nc) as tc:
        with (tc.tile_pool(name="p", bufs=1) as pool,
              tc.tile_pool(name="pd", bufs=2) as poold,
              tc.tile_pool(name="ps", bufs=2, space="PSUM") as psum,
              tc.tile_pool(name="pss", bufs=1, space="PSUM") as pss):
            ident_bf = pool.tile([128, 128], bf16, tag="ident")
            nc.gpsimd.memset(ident_bf[:], 0.0)
            nc.gpsimd.affine_select(out=ident_bf[:], in_=ident_bf[:],
                compare_op=mybir.AluOpType.not_equal, fill=1.0, base=0,
                pattern=[[-1, 128]], channel_multiplier=1)
            i7 = pool.tile([128, 128], bf16, tag="i7")
            nc.gpsimd.memset(i7[:], 0.0)
            nc.gpsimd.affine_select(out=i7[:], in_=i7[:],
                compare_op=mybir.AluOpType.not_equal, fill=7.0, base=0,
                pattern=[[-1, 128]], channel_multiplier=1)
            i15 = pool.tile([128, 128], bf16, tag="i15")
            nc.gpsimd.memset(i15[:], 0.0)
            nc.gpsimd.affine_select(out=i15[:], in_=i15[:],
                compare_op=mybir.AluOpType.not_equal, fill=15.0, base=0,
                pattern=[[-1, 128]], channel_multiplier=1)
            i13 = pool.tile([128, 128], bf16, tag="i13")
            nc.gpsimd.memset(i13[:], 0.0)
            nc.gpsimd.affine_select(out=i13[:], in_=i13[:],
                compare_op=mybir.AluOpType.not_equal, fill=13.0, base=0,
                pattern=[[-1, 128]], channel_multiplier=1)
            ones_row = pool.tile([1, 128], f32, tag="ones_row")
            nc.vector.memset(ones_row[:], 1.0)
            gs_sb = pool.tile([1, 1], f32, tag="gs_sb")
            nc.sync.dma_start(gs_sb[:], GS[:])
            ps_bc = pss.tile([128, 1], f32, tag="ps_bc")
            nc.tensor.matmul(ps_bc[:], ones_row[:], gs_sb[:], start=True, stop=True)
            gsb = pool.tile([128, 1], f32, tag="gsb")
            nc.scalar.copy(gsb[:], ps_bc[:])

            for p in range(PAIRS):
                qt_r = pool.tile([64, N], f32r, tag="qt")
                kt_r = pool.tile([64, N], f32r, tag="kt")
                nc.gpsimd.dma_start(qt_r[:], QT[p])
                nc.gpsimd.dma_start(kt_r[:], KT[p])
                v_bf = pool.tile([128, NT, 64], bf16, tag="v")
                nc.sync.dma_start(v_bf[:], Vb[p].rearrange("(t pp) d -> pp t d", pp=128))
                nct_r = pool.tile([64, M], f32r, tag="nctr")
                nrt_r = pool.tile([64, M], f32r, tag="nrtr")
                nc.gpsimd.dma_start(nct_r[:], NCT[p])
                nc.gpsimd.dma_start(nrt_r[:], NRT[p])
                nct32 = pool.tile([64, M], f32, tag="nct32")
                nrt32 = pool.tile([64, M], f32, tag="nrt32")
                nc.sync.dma_start(nct32[:], NCT[p])
                nc.sync.dma_start(nrt32[:], NRT[p])

                er = pool.tile([128, N], bf16, tag="er")
                racc = pool.tile([128, NCH], f32, tag="racc")
                for j in range(NCH):
                    ps_r = psum.tile([128, NCHUNK], f32, tag="ps_big")
                    nc.tensor.matmul(ps_r[:], nrt_r[:], kt_r[:, j*NCHUNK:(j+1)*NCHUNK],
                                     start=True, stop=True)
                    nc.scalar.activation(er[:, j*NCHUNK:(j+1)*NCHUNK], ps_r[:],
                                         AF.Exp, accum_out=racc[:, j:j+1])
                ert = pool.tile([128, NT, 128], bf16, tag="ert")
                nc.sync.dma_start_transpose(ert[:], er[:])
                ps_S = pss.tile([128, 64], f32, tag="ps_s")
                for t in range(NT):
                    nc.tensor.matmul(ps_S[:], ert[:, t, :], v_bf[:, t, :],
                                     start=(t == 0), stop=(t == NT - 1))
                rsum = pool.tile([128, 1], f32, tag="rsum")
                nc.scalar.activation(racc[:], racc[:], AF.Copy, accum_out=rsum[:])
                rrec = pool.tile([128, 1], f32, tag="rrec")
                nc.vector.reciprocal(rrec[:], rsum[:])
                s_bf = pool.tile([128, 64], bf16, tag="s_bf")
                nc.vector.tensor_scalar_mul(s_bf[:], ps_S[:], rrec[:])

                ps_m = pss.tile([128, 128], f32, tag="ps_m")
                nc.tensor.matmul(ps_m[:], nrt32[:], nct32[:], start=True, stop=True)
                e_m = pool.tile([128, 128], f32, tag="e_m")
                msum = pool.tile([128, 1], f32, tag="msum")
                nc.scalar.activation(e_m[:], ps_m[:], AF.Exp, accum_out=msum[:])
                mrec = pool.tile([128, 1], f32, tag="mrec")
                nc.vector.reciprocal(mrec[:], msum[:])
                k2_bf = pool.tile([128, 128], bf16, tag="k2")
                nc.vector.tensor_scalar_mul(k2_bf[:], e_m[:], mrec[:])

                ps_t = pss.tile([128, 128], bf16, tag="ps_m")
                nc.tensor.transpose(ps_t[:], k2_bf[:], ident_bf[:])
                k2t_bf = pool.tile([128, 128], bf16, tag="k2t")
                nc.scalar.copy(k2t_bf[:], ps_t[:])
                vm_bf = poold.tile([128, 128], bf16, tag="vm")
                nc.vector.tensor_scalar_mul(vm_bf[:], ps_t[:], gsb[:])
                for it in range(6):
                    ps_kv = pss.tile([128, 128], f32, tag="ps_m")
                    nc.tensor.matmul(ps_kv[:], k2t_bf[:], vm_bf[:], start=True, stop=True)
                    kv_bf = poold.tile([128, 128], bf16, tag="kv")
                    nc.scalar.copy(kv_bf[:], ps_kv[:])
                    t1 = poold.tile([128, 128], bf16, tag="t1")
                    nc.vector.tensor_sub(t1[:], i7[:], kv_bf[:])
                    ps_kvt = pss.tile([128, 128], bf16, tag="ps_m")
                    nc.tensor.transpose(ps_kvt[:], kv_bf[:], ident_bf[:])
                    kvt_bf = poold.tile([128, 128], bf16, tag="kvt")
                    nc.scalar.copy(kvt_bf[:], ps_kvt[:])
                    ps_t2 = pss.tile([128, 128], f32, tag="ps_m")
                    nc.tensor.matmul(ps_t2[:], kvt_bf[:], t1[:], start=True, stop=True)
                    t3 = poold.tile([128, 128], bf16, tag="t3")
                    nc.vector.tensor_sub(t3[:], i15[:], ps_t2[:])
                    ps_t4 = pss.tile([128, 128], f32, tag="ps_m")
                    nc.tensor.matmul(ps_t4[:], kvt_bf[:], t3[:], start=True, stop=True)
                    t5 = poold.tile([128, 128], bf16, tag="t5")
                    nc.vector.tensor_sub(t5[:], i13[:], ps_t4[:])
                    ps_vt = pss.tile([128, 128], bf16, tag="ps_m")
                    nc.tensor.transpose(ps_vt[:], vm_bf[:], ident_bf[:])
                    vmt_bf = poold.tile([128, 128], bf16, tag="vmt")
                    nc.scalar.copy(vmt_bf[:], ps_vt[:])
                    ps_vn = pss.tile([128, 128], f32, tag="ps_m")
                    nc.tensor.matmul(ps_vn[:], vmt_bf[:], t5[:], start=True, stop=True)
                    vm_bf = poold.tile([128, 128], bf16, tag="vm")
                    nc.vector.tensor_scalar(vm_bf[:], ps_vn[:], 0.25, scalar2=None,
                                            op0=mybir.AluOpType.mult)
                ps_vt2 = pss.tile([128, 128], bf16, tag="ps_m")
                nc.tensor.transpose(ps_vt2[:], vm_bf[:], ident_bf[:])
                vmt2 = poold.tile([128, 128], bf16, tag="vmt2")
                nc.scalar.copy(vmt2[:], ps_vt2[:])
                ps_A = pss.tile([128, 64], f32, tag="ps_a")
                nc.tensor.matmul(ps_A[:], vmt2[:], s_bf[:], start=True, stop=True)
                b_bf = pool.tile([128, 65], bf16, tag="b_bf")
                nc.vector.memset(b_bf[:, 64:65], 1.0)
                nc.vector.tensor_copy(b_bf[:, 0:64], ps_A[:])

                for j in range(NCH):
                    ps_c = psum.tile([128, NCHUNK], f32, tag="ps_big")
                    nc.tensor.matmul(ps_c[:], nct_r[:], qt_r[:, j*NCHUNK:(j+1)*NCHUNK],
                                     start=True, stop=True)
                    ec = poold.tile([128, NCHUNK], bf16, tag="ec")
                    nc.scalar.activation(ec[:], ps_c[:], AF.Exp)
                    ps_X = psum.tile([128, 4, 65], f32, tag="ps_x")
                    for t in range(4):
                        nc.tensor.matmul(ps_X[:, t, :], ec[:, t*128:(t+1)*128], b_bf[:],
                                         start=True, stop=True)
                    xrec = poold.tile([128, 4], f32, tag="xrec")
                    nc.vector.reciprocal(xrec[:], ps_X[:, :, 64])
                    xout = poold.tile([128, 4, 64], f32, tag="xout")
                    nc.vector.tensor_tensor(out=xout[:], in0=ps_X[:, :, 0:64],
                        in1=xrec.rearrange("p (t o) -> p t o", o=1).to_broadcast([128, 4, 64]),
                        op=mybir.AluOpType.mult)
                    nc.sync.dma_start(
                        XO[p, j*NCHUNK:(j+1)*NCHUNK, :].rearrange("(t pp) d -> pp t d", pp=128),
                        xout[:])
    nc.finalize()
    _cache["nc"] = nc
    return nc


def kernel(Q, K, V, mask):
    from concourse.bass_utils import run_bass_kernel_spmd

    Q = np.asarray(Q, dtype=np.float32)
    K = np.asarray(K, dtype=np.float32)
    V = np.asarray(V, dtype=np.float32)
    Qf = Q.reshape(B * H, N, D)
    Kf = K.reshape(B * H, N, D)
    Vf = V.reshape(B * H, N, D)

    nct = np.empty((B * H, D, M), np.float32)
    nrt = np.empty((B * H, D, M), np.float32)
    gmax = 0.0
    for i in range(B * H):
        for (T, out) in ((Kf, nct), (Qf, nrt)):
            s = T[i, :, 0].copy()
            s[0] = np.inf
            idx = np.argpartition(-s, M)[:M]
            out[i] = T[i, np.sort(idx), :].T
        nr = nrt[i].T.astype(np.float64)
        nc_ = nct[i].T.astype(np.float64)
        m = nr @ nc_.T
        e = np.exp(m - m.max(axis=1, keepdims=True))
        k2 = e / e.sum(axis=1, keepdims=True)
        gmax = max(gmax, float(k2.sum(axis=0).max()))

    QTf = np.ascontiguousarray(Qf.transpose(0, 2, 1))
    KTf = np.ascontiguousarray(Kf.transpose(0, 2, 1))
    Vbf = Vf.astype(ml_dtypes.bfloat16)
    gs = np.array([[1.0 / gmax]], np.float32)

    nc = _build()
    in_maps = []
    for c in range(NCORES):
        sl = slice(c * PAIRS, (c + 1) * PAIRS)
        in_maps.append({"QT": QTf[sl], "KT": KTf[sl], "Vb": Vbf[sl],
                        "NCT": nct[sl], "NRT": nrt[sl], "GS": gs})
    res = run_bass_kernel_spmd(nc, in_maps, list(range(NCORES)))
    global LAST_RESULTS
    LAST_RESULTS = res
    X = np.concatenate([res.results[c]["XO"] for c in range(NCORES)], axis=0)
    return X.reshape(B, H, N, D)



# revision 10
# speedup vs baseline: 2.1312x; 2.1312x over previous
import numpy as np
import ml_dtypes

B, H, N, D, M = 4, 12, 8192, 64, 128
NCORES = 8
PAIRS = (B * H) // NCORES   # 6 pairs per core
NG = N // 512               # 16 groups of 512 per pair

_cache = {}


def _build():
    if "nc" in _cache:
        return _cache["nc"]
    import concourse.bacc as bacc
    import concourse.mybir as mybir
    import concourse.tile as tile

    f32 = mybir.dt.float32
    f16 = mybir.dt.float16
    bf16 = mybir.dt.bfloat16
    AF = mybir.ActivationFunctionType

    nc = bacc.Bacc("TRN2", target_bir_lowering=False, debug=False)
    # K^T on partitions 0:64, Q^T on 64:128
    KQT = nc.declare_dram_parameter("KQT", [PAIRS, 128, N], f16, isOutput=False)
    V65 = nc.declare_dram_parameter("V65", [PAIRS, N, 65], bf16, isOutput=False)
    # landmark tiles: cols 0:128 = nr^T, 128:256 = nc^T
    # rows 64:128 duplicate rows 0:64 so matmuls can match either operand base
    LANDH = nc.declare_dram_parameter("LANDH", [PAIRS, 128, 256], f16, isOutput=False)
    LAND32 = nc.declare_dram_parameter("LAND32", [PAIRS, 64, 256], f32, isOutput=False)
    GS = nc.declare_dram_parameter("GS", [1, 1], f32, isOutput=False)
    # X^T output: rows 0:64 numerator, row 64 denominator
    XOT = nc.declare_dram_parameter("XOT", [PAIRS, 65, N], f32, isOutput=True)

    with tile.TileContext(nc) as tc:
        with (tc.tile_pool(name="pc", bufs=1) as pc,
              tc.tile_pool(name="pio", bufs=3) as pio,
              tc.tile_pool(name="pw", bufs=2) as pw,
              tc.tile_pool(name="pxs", bufs=2) as pxs,
              tc.tile_pool(name="pns", bufs=2) as pns,
              tc.tile_pool(name="ps_rt", bufs=2, space="PSUM") as ps_rt_pool,
              tc.tile_pool(name="ps_S", bufs=1, space="PSUM") as ps_S_pool,
              tc.tile_pool(name="ps_cm", bufs=1, space="PSUM") as ps_cm_pool,
              tc.tile_pool(name="ps_x", bufs=2, space="PSUM") as ps_x_pool,
              tc.tile_pool(name="ns_a", bufs=1, space="PSUM") as ns_a_pool,
              tc.tile_pool(name="ns_b", bufs=1, space="PSUM") as ns_b_pool):

            # ---- constants ----
            ident = pc.tile([128, 128], bf16, tag="ident")
            nc.gpsimd.memset(ident[:], 0.0)
            nc.gpsimd.affine_select(out=ident[:], in_=ident[:],
                compare_op=mybir.AluOpType.not_equal, fill=1.0, base=0,
                pattern=[[-1, 128]], channel_multiplier=1)
            diags = {}
            for val, tg in ((7.0, "i7"), (15.0, "i15"), (13.0, "i13")):
                t = pc.tile([128, 128], bf16, tag=tg)
                nc.gpsimd.memset(t[:], 0.0)
                nc.gpsimd.affine_select(out=t[:], in_=t[:],
                    compare_op=mybir.AluOpType.not_equal, fill=val, base=0,
                    pattern=[[-1, 128]], channel_multiplier=1)
                diags[tg] = t
            i7, i15, i13 = diags["i7"], diags["i15"], diags["i13"]

            ones_row = pc.tile([1, 128], f32, tag="ones_row")
            nc.vector.memset(ones_row[:], 1.0)
            gs_sb = pc.tile([1, 1], f32, tag="gs_sb")
            nc.sync.dma_start(gs_sb[:], GS[:])
            ps_bc = ps_cm_pool.tile([128, 1], f32, tag="ps_cm")
            nc.tensor.matmul(ps_bc[:], ones_row[:], gs_sb[:], start=True, stop=True)
            gsb = pc.tile([128, 1], f32, tag="gsb")
            nc.scalar.copy(gsb[:], ps_bc[:])

            # ---- landmark loads (all pairs, small) ----
            landh = []
            land32 = []
            for p in range(PAIRS):
                lh = pc.tile([128, 256], f16, tag=f"landh{p}")
                l32 = pc.tile([64, 256], f32, tag=f"land32{p}")
                nc.sync.dma_start(lh[:], LANDH[p])
                nc.sync.dma_start(l32[:], LAND32[p])
                landh.append(lh)
                land32.append(l32)

            # ---- big input loads for pair 0 ----
            kq_tiles = [None] * PAIRS
            v_tiles = [None] * PAIRS

            def load_pair(p):
                kq = pio.tile([128, N], f16, tag="kq")
                nc.gpsimd.dma_start(kq[:], KQT[p])
                vt = pio.tile([128, N // 128, 65], bf16, tag="v65")
                nc.sync.dma_start(vt[:], V65[p].rearrange("(t pp) d -> pp t d", pp=128))
                kq_tiles[p] = kq
                v_tiles[p] = vt

            load_pair(0)

            # ---- NS stage machinery (deferred per pair, drip-fed into slots) ----
            k2t_bf = [None] * PAIRS
            vm_cur = [None] * PAIRS
            vt_cur = [None] * PAIRS

            def m_chain(p):
                # k2 = softmax(nr @ nc^T); NS init state Vm0 = gs*k2^T, Vt0 = gs*k2
                ps_m = ps_cm_pool.tile([128, 128], f32, tag="ps_cm")
                nc.tensor.matmul(ps_m[:], land32[p][:, 0:128], land32[p][:, 128:256],
                                 start=True, stop=True)
                e_m = pns.tile([128, 128], f32, tag="e_m")
                msum = pns.tile([128, 1], f32, tag="msum")
                nc.scalar.activation(e_m[:], ps_m[:], AF.Exp, accum_out=msum[:])
                mrec = pns.tile([128, 1], f32, tag="mrec")
                nc.vector.reciprocal(mrec[:], msum[:])
                k2b = pns.tile([128, 128], bf16, tag="k2b")
                nc.vector.tensor_scalar_mul(k2b[:], e_m[:], mrec[:])
                ps_t = ns_a_pool.tile([128, 128], bf16, tag="ns_a")
                nc.tensor.transpose(ps_t[:], k2b[:], ident[:])
                kt = pc.tile([128, 128], bf16, tag=f"k2t{p}")
                nc.scalar.copy(kt[:], ps_t[:])
                k2t_bf[p] = kt
                vm0 = pns.tile([128, 128], bf16, tag="vm")
                nc.vector.tensor_scalar_mul(vm0[:], ps_t[:], gsb[:])
                vt0 = pns.tile([128, 128], bf16, tag="vt")
                nc.vector.tensor_scalar_mul(vt0[:], k2b[:], gsb[:])
                vm_cur[p] = vm0
                vt_cur[p] = vt0

            def ns_chunks(p):
                # generator of emission thunks: 5 chunks per iteration
                for _ in range(6):
                    def c1():
                        ps_P = ns_a_pool.tile([128, 128], f32, tag="ns_a")
                        nc.tensor.matmul(ps_P[:], k2t_bf[p][:], vm_cur[p][:],
                                         start=True, stop=True)
                        pbf = pns.tile([128, 128], bf16, tag="pbf")
                        nc.scalar.copy(pbf[:], ps_P[:])
                        t1 = pns.tile([128, 128], bf16, tag="t1")
                        nc.vector.tensor_sub(t1[:], i7[:], ps_P[:])
                        return pbf, t1
                    def c2(st):
                        pbf, t1 = st
                        ps_pt = ns_b_pool.tile([128, 128], bf16, tag="ns_b")
                        nc.tensor.transpose(ps_pt[:], pbf[:], ident[:])
                        ptb = pns.tile([128, 128], bf16, tag="ptb")
                        nc.scalar.copy(ptb[:], ps_pt[:])
                        return ptb, t1
                    def c3(st):
                        ptb, t1 = st
                        ps_u = ns_a_pool.tile([128, 128], f32, tag="ns_a")
                        nc.tensor.matmul(ps_u[:], ptb[:], t1[:], start=True, stop=True)
                        t2 = pns.tile([128, 128], bf16, tag="t2")
                        nc.vector.tensor_sub(t2[:], i15[:], ps_u[:])
                        return ptb, t2
                    def c4(st):
                        ptb, t2 = st
                        ps_w = ns_b_pool.tile([128, 128], f32, tag="ns_b")
                        nc.tensor.matmul(ps_w[:], ptb[:], t2[:], start=True, stop=True)
                        t3 = pns.tile([128, 128], bf16, tag="t3")
                        nc.vector.tensor_sub(t3[:], i13[:], ps_w[:])
                        return t3
                    def c5(st):
                        t3 = st
                        ps_v = ns_a_pool.tile([128, 128], f32, tag="ns_a")
                        nc.tensor.matmul(ps_v[:], vt_cur[p][:], t3[:], start=True, stop=True)
                        vm_n = pns.tile([128, 128], bf16, tag="vm")
                        nc.vector.tensor_scalar(vm_n[:], ps_v[:], 0.25, scalar2=None,
                                                op0=mybir.AluOpType.mult)
                        ps_vt = ns_b_pool.tile([128, 128], f32, tag="ns_b")
                        nc.tensor.matmul(ps_vt[:], t3[:], vt_cur[p][:], start=True, stop=True)
                        vt_n = pns.tile([128, 128], bf16, tag="vt")
                        nc.vector.tensor_scalar(vt_n[:], ps_vt[:], 0.25, scalar2=None,
                                                op0=mybir.AluOpType.mult)
                        vm_cur[p] = vm_n
                        vt_cur[p] = vt_n
                    yield c1, c2, c3, c4, c5

            def ns_stepper(p):
                # flat sequence of thunks threading state
                state = {"st": None}
                for c1, c2, c3, c4, c5 in ns_chunks(p):
                    yield lambda c1=c1: state.__setitem__("st", c1())
                    yield lambda c2=c2: state.__setitem__("st", c2(state["st"]))
                    yield lambda c3=c3: state.__setitem__("st", c3(state["st"]))
                    yield lambda c4=c4: state.__setitem__("st", c4(state["st"]))
                    yield lambda c5=c5: c5(state["st"])

            # ---- main pipelined slots ----
            A1 = [None] * PAIRS
            ps_S_handle = [None] * PAIRS
            ert_prev = [None]
            ec_prev = [None]
            xstage = [None]
            ps_x_prev = [None]

            for s in range(PAIRS + 1):
                if s + 1 < PAIRS:
                    load_pair(s + 1)
                if s < PAIRS:
                    m_chain(s)
                    ns_iter = ns_stepper(s)
                else:
                    ns_iter = iter(())

                for g in range(NG + 1):
                    # phase-1 r^T matmuls for pair s
                    if s < PAIRS and g < NG:
                        kq = kq_tiles[s]
                        ps_rt = ps_rt_pool.tile([128, 512], f32, tag="ps_rt")
                        for j in range(4):
                            nc.tensor.matmul(
                                ps_rt[:, j * 128:(j + 1) * 128],
                                kq[0:64, g * 512 + j * 128: g * 512 + (j + 1) * 128],
                                landh[s][0:64, 0:128],
                                start=True, stop=True)
                        ert = pw.tile([128, 512], bf16, tag="ert")
                        nc.scalar.activation(ert[:], ps_rt[:], AF.Exp)
                        ert_new = ert
                    # phase-1 S accumulation for pair s (group g-1)
                    if s < PAIRS and g >= 1:
                        if g == 1:
                            ps_S = ps_S_pool.tile([128, 65], f32, tag="ps_S")
                            ps_S_handle[s] = ps_S
                        ps_S = ps_S_handle[s]
                        for j in range(4):
                            nc.tensor.matmul(
                                ps_S[:],
                                ert_prev[0][:, j * 128:(j + 1) * 128],
                                v_tiles[s][:, (g - 1) * 4 + j, :],
                                start=(g == 1 and j == 0),
                                stop=(g == NG and j == 3),
                                skip_group_check=True)
                    if s < PAIRS and g < NG:
                        ert_prev[0] = ert_new
                    # phase-3 c matmul + exp for pair s-1
                    if s >= 1 and g < NG:
                        pprev = s - 1
                        kqp = kq_tiles[pprev]
                        ps_c = ps_cm_pool.tile([128, 512], f32, tag="ps_cm")
                        nc.tensor.matmul(ps_c[:], landh[pprev][64:128, 128:256],
                                         kqp[64:128, g * 512:(g + 1) * 512],
                                         start=True, stop=True)
                        ec = pw.tile([128, 512], bf16, tag="ec")
                        nc.scalar.activation(ec[:], ps_c[:], AF.Exp)
                        ec_new = ec
                    # phase-3 X^T matmul for pair s-1 (group g-1)
                    if s >= 1 and g >= 1:
                        pprev = s - 1
                        gm = g - 1
                        if gm % 8 == 0:
                            xstage[0] = pxs.tile([65, 4096], f32, tag="xstage",
                                                 name="xstage")
                        ps_x = ps_x_pool.tile([65, 512], f32, tag="ps_x")
                        nc.tensor.matmul(ps_x[:], A1[pprev][:], ec_prev[0][:],
                                         start=True, stop=True)
                        nc.vector.tensor_copy(
                            xstage[0][:, (gm % 8) * 512:(gm % 8 + 1) * 512], ps_x[:])
                        if gm % 8 == 7:
                            half = gm // 8
                            nc.sync.dma_start(
                                XOT[pprev, :, half * 4096:(half + 1) * 4096],
                                xstage[0][:])
                    if s >= 1 and g < NG:
                        ec_prev[0] = ec_new
                    # drip-feed NS stages (2 per g-iteration)
                    for _ in range(2):
                        th = next(ns_iter, None)
                        if th is not None:
                            th()

                # any leftover NS stages
                for th in ns_iter:
                    th()

                # A-chain for pair s: A1 = [vm @ S_norm | 1]
                if s < PAIRS:
                    ps_S = ps_S_handle[s]
                    rrec = pw.tile([128, 1], f32, tag="rrec")
                    nc.vector.reciprocal(rrec[:], ps_S[:, 64:65])
                    s_bf = pw.tile([128, 64], bf16, tag="s_bf")
                    nc.vector.tensor_scalar_mul(s_bf[:], ps_S[:, 0:64], rrec[:])
                    ps_A = ns_b_pool.tile([128, 64], f32, tag="ns_b")
                    nc.tensor.matmul(ps_A[:], vt_cur[s][:], s_bf[:], start=True, stop=True)
                    a1 = pw.tile([128, 65], bf16, tag="A1")
                    nc.vector.memset(a1[:, 64:65], 1.0)
                    nc.vector.tensor_copy(a1[:, 0:64], ps_A[:])
                    A1[s] = a1

    nc.finalize()
    _cache["nc"] = nc
    return nc


def kernel(Q, K, V, mask):
    from concourse.bass_utils import run_bass_kernel_spmd

    Q = np.asarray(Q, dtype=np.float32)
    K = np.asarray(K, dtype=np.float32)
    V = np.asarray(V, dtype=np.float32)
    BH = B * H
    Qf = Q.reshape(BH, N, D)
    Kf = K.reshape(BH, N, D)
    Vf = V.reshape(BH, N, D)

    # host: top-k selection + global NS init scalar
    landh = np.empty((BH, 128, 256), np.float16)
    land32 = np.empty((BH, 64, 256), np.float32)
    gmax = 0.0
    for i in range(BH):
        sK = Kf[i, :, 0].copy(); sK[0] = np.inf
        iK = np.sort(np.argpartition(-sK, M)[:M])
        sQ = Qf[i, :, 0].copy(); sQ[0] = np.inf
        iQ = np.sort(np.argpartition(-sQ, M)[:M])
        nr = Qf[i, iQ]          # [M, D]
        ncm = Kf[i, iK]         # [M, D]
        land32[i, :, 0:128] = nr.T
        land32[i, :, 128:256] = ncm.T
        landh[i, 0:64, 0:128] = nr.T
        landh[i, 0:64, 128:256] = ncm.T
        landh[i, 64:128, :] = landh[i, 0:64, :]
        md = nr.astype(np.float64) @ ncm.astype(np.float64).T
        e = np.exp(md - md.max(axis=1, keepdims=True))
        k2 = e / e.sum(axis=1, keepdims=True)
        gmax = max(gmax, float(k2.sum(axis=0).max()))

    kqt = np.empty((BH, 128, N), np.float16)
    kqt[:, 0:64, :] = Kf.transpose(0, 2, 1)
    kqt[:, 64:128, :] = Qf.transpose(0, 2, 1)
    v65 = np.empty((BH, N, 65), ml_dtypes.bfloat16)
    v65[:, :, 0:64] = Vf.astype(ml_dtypes.bfloat16)
    v65[:, :, 64] = 1.0
    gs = np.array([[1.0 / gmax]], np.float32)

    nc = _build()
    in_maps = []
    for c in range(NCORES):
        sl = slice(c * PAIRS, (c + 1) * PAIRS)
        in_maps.append({"KQT": kqt[sl], "V65": v65[sl],
                        "LANDH": landh[sl], "LAND32": land32[sl], "GS": gs})
    res = run_bass_kernel_spmd(nc, in_maps, list(range(NCORES)))
    global LAST_RESULTS
    LAST_RESULTS = res
    xot = np.concatenate([res.results[c]["XOT"] for c in range(NCORES)], axis=0)
    X = xot[:, 0:64, :] / xot[:, 64:65, :]
    return np.ascontiguousarray(X.transpose(0, 2, 1)).reshape(B, H, N, D).astype(np.float32)
